# revision 46
# baseline (speedup 1.0000x reference)
"""NNUE-style DeepPerspectiveNet forward pass on 8 Trainium2 NeuronCores.

Strategy: data-parallel over the batch (2048 samples per core), weights
replicated. Per core the sparse features are densified with GPSIMD
local_scatter (per-sample indexed scatter of deduplicated values), the
board is transposed via PE identity-matmuls, and all layers run as bf16
matmuls on the TensorEngine with fused bias+activation on the Scalar
engine.
"""
import sys

for _p in ("/opt/trn_rl_repo", "/root/.axon_site/_ro/trn_rl_repo"):
    if _p not in sys.path:
        sys.path.append(_p)

import numpy as np
import ml_dtypes

import concourse.bass as bass
import concourse.bacc as bacc
import concourse.tile as tile
import concourse.mybir as mybir
from concourse.bass_utils import run_bass_kernel_spmd

# Problem constants (hardcoded per the harness contract).
B = 16384          # global batch
NI = 32            # max active features per sample
NF = 768           # feature space
FT = 512           # perspective layer width
L2W = 32           # second layer width
N_CORES = 8
BL = B // N_CORES  # 2048 samples per core
TILE = 128         # samples per scatter tile
NTILE = BL // TILE         # 16
MACRO = 512                # samples per macro tile
NMACRO = BL // MACRO       # 4
TPM = MACRO // TILE        # 4 tiles per macro
NQ = NF // 128             # 6 feature chunks
NA = FT // 128             # 4 output slices of the perspective layer
NS = 2 * NA                # 8 k-slices for the L2 layer

_COMPILED = {}
RUN_KWARGS = {}
LAST_RESULT = None


def _build(zero_bp, zero_bl2):
    ZERO_BP = zero_bp
    ZERO_BL2 = zero_bl2
    dt = mybir.dt
    nc = bacc.Bacc("TRN2", target_bir_lowering=False, debug=False,
                   num_devices=N_CORES)

    def din(name, shape, dtype):
        return nc.dram_tensor(name, shape, dtype, kind="ExternalInput").ap()

    # Per-core sharded inputs. idx/val laid out [128, NTILE, NI] so tile t
    # is the partition-contiguous slice [:, t, :].
    idx_s = din("idx_s", [TILE, NTILE, NI], dt.int16)
    val_s = din("val_s", [TILE, NTILE, NI], dt.bfloat16)
    idx_n = din("idx_n", [TILE, NTILE, NI], dt.int16)
    val_n = din("val_n", [TILE, NTILE, NI], dt.bfloat16)
    # Replicated weights. wt = W_p.T reshaped [NQ, 128, FT] -> [128, NQ, FT].
    wt = din("wt", [128, NQ, FT], dt.bfloat16)
    wl2 = din("wl2", [128, NS, L2W], dt.bfloat16)   # W_l2.T as [128,8,32]
    wout = din("wout", [L2W, 1], dt.bfloat16)       # W_out.T
    bp = din("bp", [128, NA], dt.float32)           # b_p as [128,4]
    bl2 = din("bl2", [L2W, 1], dt.float32)
    bout = din("bout", [1, 1], dt.float32)
    ident = din("ident", [128, 128], dt.bfloat16)

    out_ap = nc.dram_tensor("out", [1, BL], dt.float32,
                            kind="ExternalOutput").ap()

    with tile.TileContext(nc) as tc:
        with (
            tc.tile_pool(name="const", bufs=1) as cpool,
            tc.tile_pool(name="hpool", bufs=3 * TPM) as hpool,
            tc.tile_pool(name="tq", bufs=3) as tqpool,
            tc.tile_pool(name="xpool", bufs=3) as xpool,
            tc.tile_pool(name="x2pool", bufs=2) as x2pool,
            tc.tile_pool(name="xcpool", bufs=4) as xcpool,
            tc.tile_pool(name="opool", bufs=2) as opool,
            tc.tile_pool(name="ps_t", bufs=3, space=bass.MemorySpace.PSUM) as ps_t,
            tc.tile_pool(name="ps_pov", bufs=2, space=bass.MemorySpace.PSUM) as ps_pov,
            tc.tile_pool(name="ps_head", bufs=1, space=bass.MemorySpace.PSUM) as ps_head,
        ):
            # --- warmup: pull the GPSIMD ext-isa library into IRAM and
            # the ACT LUTs before the first real dependency chain ---
            wu = cpool.tile([128, 2], dt.bfloat16, tag="wu")
            wu_i = cpool.tile([128, 2], dt.int16, tag="wu_i")
            nc.vector.memset(wu_i[:], -1)
            nc.vector.memset(wu[:], 0)
            nc.gpsimd.local_scatter(wu[:], wu[:], wu_i[:],
                                    channels=128, num_elems=2, num_idxs=2)
            wu_a = cpool.tile([128, 2], dt.float32, tag="wu_a")
            nc.scalar.activation(wu_a[:], wu[:],
                                 mybir.ActivationFunctionType.Relu)
            nc.scalar.activation(wu_a[:], wu[:],
                                 mybir.ActivationFunctionType.Sigmoid)
            # PE warm-up: ~3 us of dummy matmuls on scratch data releases
            # the HAM clock throttle before the first real matmul arrives.
            wu_m = cpool.tile([128, 512], dt.bfloat16, tag="wu_m")
            nc.vector.memset(wu_m[:], 0)
            wu_ps = ps_head.tile([128, 512], dt.float32,
                                 name="wups", tag="head")
            for i in range(14):
                nc.tensor.matmul(wu_ps[:], wu_m[:, 0:128], wu_m[:],
                                 start=True, stop=True)

            # --- load constants/weights/indices once. Launch order and
            # queue split matter: each DMA launch occupies its queue for
            # ~600 ns, and the identity + idx/val gate the first matmuls.
            id_t = cpool.tile([128, 128], dt.bfloat16, tag="ident")
            nc.scalar.dma_start(id_t[:], ident[:, :])
            idx_t = {}
            val_t = {}
            for pov, (ia, va) in enumerate(((idx_s, val_s), (idx_n, val_n))):
                idx_t[pov] = cpool.tile([TILE, NTILE, NI], dt.int16,
                                        name=f"idx{pov}", tag=f"idx{pov}")
                val_t[pov] = cpool.tile([TILE, NTILE, NI], dt.bfloat16,
                                        name=f"val{pov}", tag=f"val{pov}")
                if pov == 0:
                    nc.sync.dma_start(idx_t[pov][:, 0:1, :], ia[:, 0:1, :])
                    nc.sync.dma_start(val_t[pov][:, 0:1, :], va[:, 0:1, :])
                    nc.sync.dma_start(idx_t[pov][:, 1:, :], ia[:, 1:, :])
                    nc.sync.dma_start(val_t[pov][:, 1:, :], va[:, 1:, :])
                else:
                    nc.sync.dma_start(idx_t[pov][:], ia[:, :, :])
                    nc.sync.dma_start(val_t[pov][:], va[:, :, :])
            wt_t = cpool.tile([128, NQ, FT], dt.bfloat16, tag="wt")
            nc.scalar.dma_start(wt_t[:], wt[:, :, :])
            wl2_t = cpool.tile([128, NS, L2W], dt.bfloat16, tag="wl2")
            nc.scalar.dma_start(wl2_t[:], wl2[:, :, :])
            wout_t = cpool.tile([L2W, 1], dt.bfloat16, tag="wout")
            nc.scalar.dma_start(wout_t[:], wout[:, :])
            bp_t = cpool.tile([128, NA], dt.float32, tag="bp")
            nc.scalar.dma_start(bp_t[:], bp[:, :])
            bl2_t = cpool.tile([L2W, 1], dt.float32, tag="bl2")
            nc.scalar.dma_start(bl2_t[:], bl2[:, :])
            bout_t = cpool.tile([1, 1], dt.float32, tag="bout")
            nc.scalar.dma_start(bout_t[:], bout[:, :])

            # Processing units as (first 128-sample tile, tile count).
            UNITS = [(0, 4), (4, 4), (8, 4), (12, 4)]
            x_ts = {}

            def head(u):
                # L2 + output layers + sigmoid for unit u (runs deferred,
                # overlapped with the next unit's perspective work).
                tile0, ntiles = UNITS[u]
                w = ntiles * TILE
                x_t = x_ts.pop(u)
                l2_ps = ps_head.tile([L2W, MACRO], dt.float32,
                                     name="l2ps", tag="head")
                for s in range(NS):
                    nc.tensor.matmul(
                        l2_ps[:, 0:w], wl2_t[:, s, :], x_t[:, s, :],
                        start=(s == 0), stop=(s == NS - 1),
                    )
                x2_t = x2pool.tile([L2W, MACRO], dt.bfloat16, tag="x2")
                if ZERO_BL2:
                    nc.vector.tensor_scalar(
                        x2_t[:, 0:w], l2_ps[:, 0:w], 1.0, 0.0,
                        mybir.AluOpType.min, mybir.AluOpType.max,
                    )
                else:
                    nc.scalar.activation(
                        x2_t[:, 0:w], l2_ps[:, 0:w],
                        mybir.ActivationFunctionType.Relu,
                        bias=bl2_t[:, 0:1],
                    )
                    nc.vector.tensor_scalar_min(x2_t[:, 0:w],
                                                x2_t[:, 0:w], 1.0)
                o_ps = ps_head.tile([1, MACRO], dt.float32,
                                    name="ops", tag="head")
                nc.tensor.matmul(o_ps[:, 0:w], wout_t[:], x2_t[:, 0:w],
                                 start=True, stop=True)
                o_sb = opool.tile([1, MACRO], dt.float32, tag="osb")
                nc.scalar.activation(
                    o_sb[:, 0:w], o_ps[:, 0:w],
                    mybir.ActivationFunctionType.Sigmoid,
                    bias=bout_t[:, 0:1],
                )
                b0 = tile0 * TILE
                nc.sync.dma_start(out_ap[:, b0:b0 + w], o_sb[:, 0:w])

            for u, (tile0, ntiles) in enumerate(UNITS):
                w = ntiles * TILE
                # X holds the 8 activated k-slices for this unit:
                # slices 0..3 = stm perspective, 4..7 = nstm.
                x_t = xpool.tile([128, NS, MACRO], dt.bfloat16, tag="x")
                x_t = x_t[:, :, 0:w]
                x_ts[u] = x_t
                if u >= 1:
                    head(u - 1)
                for pov in range(2):
                    # --- densify: H[t] = board tile [128 samples, NF] ---
                    hts = []
                    for t in range(ntiles):
                        gt = tile0 + t
                        ht = hpool.tile([TILE, NF], dt.bfloat16,
                                        name=f"h{t}", tag=f"h{t}")
                        nc.gpsimd.local_scatter(
                            ht[:], val_t[pov][:, gt, :], idx_t[pov][:, gt, :],
                            channels=TILE, num_elems=NF, num_idxs=NI,
                        )
                        hts.append(ht)

                    # --- perspective layer: tiles 0..n-2 are transposed
                    # via PE identity matmuls + bf16 casts (split across
                    # Scalar and Vector); the last tile goes through the
                    # DMA crossbar on the otherwise-idle Sync queue (its
                    # row mapping f -> partition f%128, plane f//128 is
                    # exactly the q-chunk layout). t_all is t-major so the
                    # perspective matmul reads all tiles as one strided AP.
                    npe = ntiles - 1
                    t_all = tqpool.tile([128, TPM, NQ, TILE], dt.bfloat16,
                                        name="tall", tag="tall")
                    nc.sync.dma_start_transpose(
                        t_all[:, npe, :, :], hts[npe][:])
                    for q in range(NQ):
                        t_ps = ps_t.tile([128, TPM, TILE], dt.float32,
                                         tag="tps")
                        for t in range(npe):
                            nc.tensor.matmul(
                                t_ps[:, t, :],
                                hts[t][:, q * 128:(q + 1) * 128],
                                id_t[:],
                                start=True, stop=True,
                            )
                        if q % 2 == 0:
                            nc.scalar.copy(t_all[:, 0:npe, q, :],
                                           t_ps[:, 0:npe, :])
                        else:
                            nc.vector.tensor_copy(t_all[:, 0:npe, q, :],
                                                  t_ps[:, 0:npe, :])

                    for g in range(2):
                        pv_ps = ps_pov.tile([128, 2, MACRO], dt.float32,
                                            name="pv", tag="pv")
                        for q in range(NQ):
                            for al in range(2):
                                a = g * 2 + al
                                nc.tensor.matmul(
                                    pv_ps[:, al, 0:w],
                                    wt_t[:, q, a * 128:(a + 1) * 128],
                                    t_all[:, 0:ntiles, q, :],
                                    start=(q == 0), stop=(q == NQ - 1),
                                )
                        xw = x_t[:, pov * NA + g * 2: pov * NA + g * 2 + 2, :]
                        if ZERO_BP:
                            xc = xcpool.tile([128, 2, MACRO], dt.bfloat16,
                                             name="xc", tag="xc")
                            nc.vector.tensor_scalar(
                                xc[:, :, 0:w], pv_ps[:, :, 0:w], 1.0, 0.0,
                                mybir.AluOpType.min, mybir.AluOpType.max,
                            )
                            nc.scalar.activation(
                                xw, xc[:, :, 0:w],
                                mybir.ActivationFunctionType.Square,
                            )
                        else:
                            for al in range(2):
                                a = g * 2 + al
                                xs = x_t[:, pov * NA + a, :]
                                nc.scalar.activation(
                                    xs, pv_ps[:, al, 0:w],
                                    mybir.ActivationFunctionType.Relu,
                                    bias=bp_t[:, a:a + 1],
                                )
                                nc.vector.tensor_scalar_min(xs, xs, 1.0)
                                nc.vector.tensor_mul(xs, xs, xs)

            head(len(UNITS) - 1)

    nc.compile()
    return nc


def _get_compiled(zero_bp, zero_bl2):
    key = (zero_bp, zero_bl2)
    if key not in _COMPILED:
        _COMPILED[key] = _build(zero_bp, zero_bl2)
    return _COMPILED[key]


def _dedup_rows(feats, vals):
    """Merge duplicate feature ids within each row.

    feats: [N, NI] int, vals: [N, NI] float32.
    Returns (idx int16 with -1 padding, val float32) where each feature id
    appears at most once per row with its values summed.
    """
    n = feats.shape[0]
    order = np.argsort(feats, axis=1, kind="stable")
    fs = np.take_along_axis(feats, order, 1)
    vs = np.take_along_axis(vals, order, 1)
    new_run = np.ones((n, NI), dtype=bool)
    new_run[:, 1:] = fs[:, 1:] != fs[:, :-1]
    run_id = np.cumsum(new_run, axis=1) - 1          # [N, NI] in [0, NI)
    rows = np.repeat(np.arange(n, dtype=np.int64), NI)
    key = rows * NI + run_id.ravel()
    totals = np.bincount(key, weights=vs.ravel().astype(np.float64),
                         minlength=n * NI).reshape(n, NI)
    idx = np.where(new_run, fs, -1).astype(np.int16)
    val = np.where(new_run, totals[np.arange(n)[:, None], run_id], 0.0)
    return idx, val.astype(np.float32)


def _prep_sparse(indices, values):
    """Interleaved (sample, feature) pairs -> per-core scatter inputs."""
    pairs = np.asarray(indices).astype(np.int64).reshape(-1, 2)
    vals = np.asarray(values, dtype=np.float32)
    samp = pairs[:, 0]
    expect = np.repeat(np.arange(B, dtype=np.int64), NI)
    if not np.array_equal(samp, expect):
        # general layout: stable-sort nnz by sample id first
        order = np.argsort(samp, kind="stable")
        pairs = pairs[order]
        vals = vals[order]
        assert np.array_equal(pairs[:, 0], expect), \
            "each sample must have exactly NI (sample, feature) pairs"
    feats = pairs[:, 1].reshape(B, NI)
    vals = vals.reshape(B, NI)
    idx, val = _dedup_rows(feats, vals)
    # [B, NI] -> per-core [TILE, NTILE, NI] with sample = t*TILE + p
    idx = idx.reshape(N_CORES, NTILE, TILE, NI).transpose(0, 2, 1, 3)
    val = val.reshape(N_CORES, NTILE, TILE, NI).transpose(0, 2, 1, 3)
    return (np.ascontiguousarray(idx),
            np.ascontiguousarray(val.astype(ml_dtypes.bfloat16)))


def kernel(stm_indices, nstm_indices, values, size,
           W_p, b_p, W_l2, b_l2, W_out, b_out):
    assert int(size) == B
    zero_bp = not np.any(np.asarray(b_p))
    zero_bl2 = not np.any(np.asarray(b_l2))
    nc = _get_compiled(zero_bp, zero_bl2)

    idx_s, val_s = _prep_sparse(stm_indices, values)
    idx_n, val_n = _prep_sparse(nstm_indices, values)

    bf16 = ml_dtypes.bfloat16
    W_p = np.asarray(W_p, dtype=np.float32)       # [FT, NF]
    wt = np.ascontiguousarray(
        W_p.T.reshape(NQ, 128, FT).transpose(1, 0, 2).astype(bf16))
    W_l2 = np.asarray(W_l2, dtype=np.float32)     # [L2W, 2*FT]
    wl2 = np.ascontiguousarray(
        W_l2.T.reshape(NS, 128, L2W).transpose(1, 0, 2).astype(bf16))
    wout = np.ascontiguousarray(
        np.asarray(W_out, dtype=np.float32).T.astype(bf16))  # [32,1]
    bp = np.ascontiguousarray(
        np.asarray(b_p, dtype=np.float32).reshape(NA, 128).T)
    bl2 = np.asarray(b_l2, dtype=np.float32).reshape(L2W, 1)
    bout = np.asarray(b_out, dtype=np.float32).reshape(1, 1)
    ident = np.eye(128, dtype=bf16)

    in_maps = []
    for c in range(N_CORES):
        in_maps.append({
            "idx_s": idx_s[c], "val_s": val_s[c],
            "idx_n": idx_n[c], "val_n": val_n[c],
            "wt": wt, "wl2": wl2, "wout": wout,
            "bp": bp, "bl2": bl2, "bout": bout,
            "ident": ident,
        })

    res = run_bass_kernel_spmd(nc, in_maps, core_ids=list(range(N_CORES)),
                               **RUN_KWARGS)
    global LAST_RESULT
    LAST_RESULT = res
    out = np.concatenate([res.results[c]["out"].reshape(BL)
                          for c in range(N_CORES)])
    return out.reshape(B, 1).astype(np.float32)


# revision 47
# speedup vs baseline: 1.0688x; 1.0688x over previous
"""NNUE-style DeepPerspectiveNet forward pass on 8 Trainium2 NeuronCores.

Strategy: data-parallel over the batch (2048 samples per core), weights
replicated. Per core the sparse features are densified with GPSIMD
local_scatter (per-sample indexed scatter of deduplicated values), the
board is transposed via PE identity-matmuls, and all layers run as bf16
matmuls on the TensorEngine with fused bias+activation on the Scalar
engine.
"""
import sys

for _p in ("/opt/trn_rl_repo", "/root/.axon_site/_ro/trn_rl_repo"):
    if _p not in sys.path:
        sys.path.append(_p)

import numpy as np
import ml_dtypes

import concourse.bass as bass
import concourse.bacc as bacc
import concourse.tile as tile
import concourse.mybir as mybir
from concourse.bass_utils import run_bass_kernel_spmd

# Problem constants (hardcoded per the harness contract).
B = 16384          # global batch
NI = 32            # max active features per sample
NF = 768           # feature space
FT = 512           # perspective layer width
L2W = 32           # second layer width
N_CORES = 8
BL = B // N_CORES  # 2048 samples per core
TILE = 128         # samples per scatter tile
NTILE = BL // TILE         # 16
MACRO = 512                # samples per macro tile
NMACRO = BL // MACRO       # 4
TPM = MACRO // TILE        # 4 tiles per macro
NQ = NF // 128             # 6 feature chunks
NA = FT // 128             # 4 output slices of the perspective layer
NS = 2 * NA                # 8 k-slices for the L2 layer

_COMPILED = {}
RUN_KWARGS = {}
LAST_RESULT = None


def _build(zero_bp, zero_bl2):
    ZERO_BP = zero_bp
    ZERO_BL2 = zero_bl2
    dt = mybir.dt
    nc = bacc.Bacc("TRN2", target_bir_lowering=False, debug=False,
                   num_devices=N_CORES)

    def din(name, shape, dtype):
        return nc.dram_tensor(name, shape, dtype, kind="ExternalInput").ap()

    # Per-core sharded inputs. idx/val laid out [128, NTILE, NI] so tile t
    # is the partition-contiguous slice [:, t, :].
    idx_s = din("idx_s", [TILE, NTILE, NI], dt.int16)
    val_s = din("val_s", [TILE, NTILE, NI], dt.bfloat16)
    idx_n = din("idx_n", [TILE, NTILE, NI], dt.int16)
    val_n = din("val_n", [TILE, NTILE, NI], dt.bfloat16)
    # Replicated weights. wt = W_p.T reshaped [NQ, 128, FT] -> [128, NQ, FT].
    wt = din("wt", [128, NQ, FT], dt.bfloat16)
    wl2 = din("wl2", [128, NS, L2W], dt.bfloat16)   # W_l2.T as [128,8,32]
    wout = din("wout", [L2W, 1], dt.bfloat16)       # W_out.T
    bp = din("bp", [128, NA], dt.float32)           # b_p as [128,4]
    bl2 = din("bl2", [L2W, 1], dt.float32)
    bout = din("bout", [1, 1], dt.float32)
    ident = din("ident", [128, 128], dt.bfloat16)

    out_ap = nc.dram_tensor("out", [1, BL], dt.float32,
                            kind="ExternalOutput").ap()

    with tile.TileContext(nc) as tc:
        with (
            tc.tile_pool(name="const", bufs=1) as cpool,
            tc.tile_pool(name="hpool", bufs=3 * TPM) as hpool,
            tc.tile_pool(name="tq", bufs=10) as tqpool,
            tc.tile_pool(name="xpool", bufs=3) as xpool,
            tc.tile_pool(name="x2pool", bufs=2) as x2pool,
            tc.tile_pool(name="xcpool", bufs=4) as xcpool,
            tc.tile_pool(name="opool", bufs=2) as opool,
            tc.tile_pool(name="ps_t", bufs=3, space=bass.MemorySpace.PSUM) as ps_t,
            tc.tile_pool(name="ps_pov", bufs=2, space=bass.MemorySpace.PSUM) as ps_pov,
            tc.tile_pool(name="ps_head", bufs=1, space=bass.MemorySpace.PSUM) as ps_head,
        ):
            # --- warmup: pull the GPSIMD ext-isa library into IRAM and
            # the ACT LUTs before the first real dependency chain ---
            wu = cpool.tile([128, 2], dt.bfloat16, tag="wu")
            wu_i = cpool.tile([128, 2], dt.int16, tag="wu_i")
            nc.vector.memset(wu_i[:], -1)
            nc.vector.memset(wu[:], 0)
            nc.gpsimd.local_scatter(wu[:], wu[:], wu_i[:],
                                    channels=128, num_elems=2, num_idxs=2)
            wu_a = cpool.tile([128, 2], dt.float32, tag="wu_a")
            nc.scalar.activation(wu_a[:], wu[:],
                                 mybir.ActivationFunctionType.Relu)
            nc.scalar.activation(wu_a[:], wu[:],
                                 mybir.ActivationFunctionType.Sigmoid)
            # PE warm-up: ~3 us of dummy matmuls on scratch data releases
            # the HAM clock throttle before the first real matmul arrives.
            wu_m = cpool.tile([128, 512], dt.bfloat16, tag="wu_m")
            nc.vector.memset(wu_m[:], 0)
            wu_ps = ps_head.tile([128, 512], dt.float32,
                                 name="wups", tag="head")
            for i in range(14):
                nc.tensor.matmul(wu_ps[:], wu_m[:, 0:128], wu_m[:],
                                 start=True, stop=True)

            # --- load constants/weights/indices once. Launch order and
            # queue split matter: each DMA launch occupies its queue for
            # ~600 ns, and the identity + idx/val gate the first matmuls.
            id_t = cpool.tile([128, 128], dt.bfloat16, tag="ident")
            nc.scalar.dma_start(id_t[:], ident[:, :])
            idx_t = {}
            val_t = {}
            for pov, (ia, va) in enumerate(((idx_s, val_s), (idx_n, val_n))):
                idx_t[pov] = cpool.tile([TILE, NTILE, NI], dt.int16,
                                        name=f"idx{pov}", tag=f"idx{pov}")
                val_t[pov] = cpool.tile([TILE, NTILE, NI], dt.bfloat16,
                                        name=f"val{pov}", tag=f"val{pov}")
                if pov == 0:
                    nc.sync.dma_start(idx_t[pov][:, 0:1, :], ia[:, 0:1, :])
                    nc.sync.dma_start(val_t[pov][:, 0:1, :], va[:, 0:1, :])
                    nc.sync.dma_start(idx_t[pov][:, 1:, :], ia[:, 1:, :])
                    nc.sync.dma_start(val_t[pov][:, 1:, :], va[:, 1:, :])
                else:
                    nc.sync.dma_start(idx_t[pov][:], ia[:, :, :])
                    nc.sync.dma_start(val_t[pov][:], va[:, :, :])
            wt_t = cpool.tile([128, NQ, FT], dt.bfloat16, tag="wt")
            nc.scalar.dma_start(wt_t[:], wt[:, :, :])
            wl2_t = cpool.tile([128, NS, L2W], dt.bfloat16, tag="wl2")
            nc.scalar.dma_start(wl2_t[:], wl2[:, :, :])
            wout_t = cpool.tile([L2W, 1], dt.bfloat16, tag="wout")
            nc.scalar.dma_start(wout_t[:], wout[:, :])
            bp_t = cpool.tile([128, NA], dt.float32, tag="bp")
            nc.scalar.dma_start(bp_t[:], bp[:, :])
            bl2_t = cpool.tile([L2W, 1], dt.float32, tag="bl2")
            nc.scalar.dma_start(bl2_t[:], bl2[:, :])
            bout_t = cpool.tile([1, 1], dt.float32, tag="bout")
            nc.scalar.dma_start(bout_t[:], bout[:, :])

            # Processing units as (first 128-sample tile, tile count).
            UNITS = [(0, 4), (4, 4), (8, 4), (12, 4)]
            x_ts = {}

            def head(u):
                # L2 + output layers + sigmoid for unit u (runs deferred,
                # overlapped with the next unit's perspective work).
                tile0, ntiles = UNITS[u]
                w = ntiles * TILE
                x_t = x_ts.pop(u)
                l2_ps = ps_head.tile([L2W, MACRO], dt.float32,
                                     name="l2ps", tag="head")
                for s in range(NS):
                    nc.tensor.matmul(
                        l2_ps[:, 0:w], wl2_t[:, s, :], x_t[:, s, :],
                        start=(s == 0), stop=(s == NS - 1),
                    )
                x2_t = x2pool.tile([L2W, MACRO], dt.bfloat16, tag="x2")
                if ZERO_BL2:
                    nc.vector.tensor_scalar(
                        x2_t[:, 0:w], l2_ps[:, 0:w], 1.0, 0.0,
                        mybir.AluOpType.min, mybir.AluOpType.max,
                    )
                else:
                    nc.scalar.activation(
                        x2_t[:, 0:w], l2_ps[:, 0:w],
                        mybir.ActivationFunctionType.Relu,
                        bias=bl2_t[:, 0:1],
                    )
                    nc.vector.tensor_scalar_min(x2_t[:, 0:w],
                                                x2_t[:, 0:w], 1.0)
                o_ps = ps_head.tile([1, MACRO], dt.float32,
                                    name="ops", tag="head")
                nc.tensor.matmul(o_ps[:, 0:w], wout_t[:], x2_t[:, 0:w],
                                 start=True, stop=True)
                o_sb = opool.tile([1, MACRO], dt.float32, tag="osb")
                nc.scalar.activation(
                    o_sb[:, 0:w], o_ps[:, 0:w],
                    mybir.ActivationFunctionType.Sigmoid,
                    bias=bout_t[:, 0:1],
                )
                b0 = tile0 * TILE
                nc.sync.dma_start(out_ap[:, b0:b0 + w], o_sb[:, 0:w])

            for u, (tile0, ntiles) in enumerate(UNITS):
                w = ntiles * TILE
                # X holds the 8 activated k-slices for this unit:
                # slices 0..3 = stm perspective, 4..7 = nstm.
                x_t = xpool.tile([128, NS, MACRO], dt.bfloat16, tag="x")
                x_t = x_t[:, :, 0:w]
                x_ts[u] = x_t
                if u >= 1:
                    head(u - 1)
                for pov in range(2):
                    # --- densify: H[t] = board tile [128 samples, NF] ---
                    hts = []
                    for t in range(ntiles):
                        gt = tile0 + t
                        ht = hpool.tile([TILE, NF], dt.bfloat16,
                                        name=f"h{t}", tag=f"h{t}")
                        nc.gpsimd.local_scatter(
                            ht[:], val_t[pov][:, gt, :], idx_t[pov][:, gt, :],
                            channels=TILE, num_elems=NF, num_idxs=NI,
                        )
                        hts.append(ht)

                    # --- perspective layer: transpose each 128-wide
                    # feature chunk via identity matmul, cast to bf16
                    # (split across Scalar and Vector), then accumulate
                    # W_p.T in two 2-bank PSUM groups so consecutive units
                    # overlap.
                    t_sbs = []
                    for q in range(NQ):
                        t_ps = ps_t.tile([128, MACRO], dt.float32, tag="tps")
                        for t in range(ntiles):
                            nc.tensor.matmul(
                                t_ps[:, t * TILE:(t + 1) * TILE],
                                hts[t][:, q * 128:(q + 1) * 128],
                                id_t[:],
                                start=True, stop=True,
                            )
                        t_sb = tqpool.tile([128, MACRO], dt.bfloat16,
                                           name="tsb", tag="tsb")
                        if q % 2 == 0:
                            nc.scalar.copy(t_sb[:, 0:w], t_ps[:, 0:w])
                        else:
                            nc.vector.tensor_copy(t_sb[:, 0:w], t_ps[:, 0:w])
                        t_sbs.append(t_sb)

                    for g in range(2):
                        pv_ps = ps_pov.tile([128, 2, MACRO], dt.float32,
                                            name="pv", tag="pv")
                        for q in range(NQ):
                            for al in range(2):
                                a = g * 2 + al
                                nc.tensor.matmul(
                                    pv_ps[:, al, 0:w],
                                    wt_t[:, q, a * 128:(a + 1) * 128],
                                    t_sbs[q][:, 0:w],
                                    start=(q == 0), stop=(q == NQ - 1),
                                )
                        xw = x_t[:, pov * NA + g * 2: pov * NA + g * 2 + 2, :]
                        if ZERO_BP:
                            xc = xcpool.tile([128, 2, MACRO], dt.bfloat16,
                                             name="xc", tag="xc")
                            nc.vector.tensor_scalar(
                                xc[:, :, 0:w], pv_ps[:, :, 0:w], 1.0, 0.0,
                                mybir.AluOpType.min, mybir.AluOpType.max,
                            )
                            nc.scalar.activation(
                                xw, xc[:, :, 0:w],
                                mybir.ActivationFunctionType.Square,
                            )
                        else:
                            for al in range(2):
                                a = g * 2 + al
                                xs = x_t[:, pov * NA + a, :]
                                nc.scalar.activation(
                                    xs, pv_ps[:, al, 0:w],
                                    mybir.ActivationFunctionType.Relu,
                                    bias=bp_t[:, a:a + 1],
                                )
                                nc.vector.tensor_scalar_min(xs, xs, 1.0)
                                nc.vector.tensor_mul(xs, xs, xs)

            head(len(UNITS) - 1)

    nc.compile()
    return nc


def _get_compiled(zero_bp, zero_bl2):
    key = (zero_bp, zero_bl2)
    if key not in _COMPILED:
        _COMPILED[key] = _build(zero_bp, zero_bl2)
    return _COMPILED[key]


def _dedup_rows(feats, vals):
    """Merge duplicate feature ids within each row.

    feats: [N, NI] int, vals: [N, NI] float32.
    Returns (idx int16 with -1 padding, val float32) where each feature id
    appears at most once per row with its values summed.
    """
    n = feats.shape[0]
    order = np.argsort(feats, axis=1, kind="stable")
    fs = np.take_along_axis(feats, order, 1)
    vs = np.take_along_axis(vals, order, 1)
    new_run = np.ones((n, NI), dtype=bool)
    new_run[:, 1:] = fs[:, 1:] != fs[:, :-1]
    run_id = np.cumsum(new_run, axis=1) - 1          # [N, NI] in [0, NI)
    rows = np.repeat(np.arange(n, dtype=np.int64), NI)
    key = rows * NI + run_id.ravel()
    totals = np.bincount(key, weights=vs.ravel().astype(np.float64),
                         minlength=n * NI).reshape(n, NI)
    idx = np.where(new_run, fs, -1).astype(np.int16)
    val = np.where(new_run, totals[np.arange(n)[:, None], run_id], 0.0)
    return idx, val.astype(np.float32)


def _prep_sparse(indices, values):
    """Interleaved (sample, feature) pairs -> per-core scatter inputs."""
    pairs = np.asarray(indices).astype(np.int64).reshape(-1, 2)
    vals = np.asarray(values, dtype=np.float32)
    samp = pairs[:, 0]
    expect = np.repeat(np.arange(B, dtype=np.int64), NI)
    if not np.array_equal(samp, expect):
        # general layout: stable-sort nnz by sample id first
        order = np.argsort(samp, kind="stable")
        pairs = pairs[order]
        vals = vals[order]
        assert np.array_equal(pairs[:, 0], expect), \
            "each sample must have exactly NI (sample, feature) pairs"
    feats = pairs[:, 1].reshape(B, NI)
    vals = vals.reshape(B, NI)
    idx, val = _dedup_rows(feats, vals)
    # [B, NI] -> per-core [TILE, NTILE, NI] with sample = t*TILE + p
    idx = idx.reshape(N_CORES, NTILE, TILE, NI).transpose(0, 2, 1, 3)
    val = val.reshape(N_CORES, NTILE, TILE, NI).transpose(0, 2, 1, 3)
    return (np.ascontiguousarray(idx),
            np.ascontiguousarray(val.astype(ml_dtypes.bfloat16)))


def kernel(stm_indices, nstm_indices, values, size,
           W_p, b_p, W_l2, b_l2, W_out, b_out):
    assert int(size) == B
    zero_bp = not np.any(np.asarray(b_p))
    zero_bl2 = not np.any(np.asarray(b_l2))
    nc = _get_compiled(zero_bp, zero_bl2)

    idx_s, val_s = _prep_sparse(stm_indices, values)
    idx_n, val_n = _prep_sparse(nstm_indices, values)

    bf16 = ml_dtypes.bfloat16
    W_p = np.asarray(W_p, dtype=np.float32)       # [FT, NF]
    wt = np.ascontiguousarray(
        W_p.T.reshape(NQ, 128, FT).transpose(1, 0, 2).astype(bf16))
    W_l2 = np.asarray(W_l2, dtype=np.float32)     # [L2W, 2*FT]
    wl2 = np.ascontiguousarray(
        W_l2.T.reshape(NS, 128, L2W).transpose(1, 0, 2).astype(bf16))
    wout = np.ascontiguousarray(
        np.asarray(W_out, dtype=np.float32).T.astype(bf16))  # [32,1]
    bp = np.ascontiguousarray(
        np.asarray(b_p, dtype=np.float32).reshape(NA, 128).T)
    bl2 = np.asarray(b_l2, dtype=np.float32).reshape(L2W, 1)
    bout = np.asarray(b_out, dtype=np.float32).reshape(1, 1)
    ident = np.eye(128, dtype=bf16)

    in_maps = []
    for c in range(N_CORES):
        in_maps.append({
            "idx_s": idx_s[c], "val_s": val_s[c],
            "idx_n": idx_n[c], "val_n": val_n[c],
            "wt": wt, "wl2": wl2, "wout": wout,
            "bp": bp, "bl2": bl2, "bout": bout,
            "ident": ident,
        })

    res = run_bass_kernel_spmd(nc, in_maps, core_ids=list(range(N_CORES)),
                               **RUN_KWARGS)
    global LAST_RESULT
    LAST_RESULT = res
    out = np.concatenate([res.results[c]["out"].reshape(BL)
                          for c in range(N_CORES)])
    return out.reshape(B, 1).astype(np.float32)


# revision 48
# speedup vs baseline: 1.0874x; 1.0173x over previous
"""NNUE-style DeepPerspectiveNet forward pass on 8 Trainium2 NeuronCores.

Strategy: data-parallel over the batch (2048 samples per core), weights
replicated. Per core the sparse features are densified with GPSIMD
local_scatter (per-sample indexed scatter of deduplicated values), the
board is transposed via PE identity-matmuls, and all layers run as bf16
matmuls on the TensorEngine with fused bias+activation on the Scalar
engine.
"""
import sys

for _p in ("/opt/trn_rl_repo", "/root/.axon_site/_ro/trn_rl_repo"):
    if _p not in sys.path:
        sys.path.append(_p)

import numpy as np
import ml_dtypes

import concourse.bass as bass
import concourse.bacc as bacc
import concourse.tile as tile
import concourse.mybir as mybir
from concourse.bass_utils import run_bass_kernel_spmd

# Problem constants (hardcoded per the harness contract).
B = 16384          # global batch
NI = 32            # max active features per sample
NF = 768           # feature space
FT = 512           # perspective layer width
L2W = 32           # second layer width
N_CORES = 8
BL = B // N_CORES  # 2048 samples per core
TILE = 128         # samples per scatter tile
NTILE = BL // TILE         # 16
MACRO = 512                # samples per macro tile
NMACRO = BL // MACRO       # 4
TPM = MACRO // TILE        # 4 tiles per macro
NQ = NF // 128             # 6 feature chunks
NA = FT // 128             # 4 output slices of the perspective layer
NS = 2 * NA                # 8 k-slices for the L2 layer

_COMPILED = {}
RUN_KWARGS = {}
LAST_RESULT = None


def _build(zero_bp, zero_bl2):
    ZERO_BP = zero_bp
    ZERO_BL2 = zero_bl2
    dt = mybir.dt
    nc = bacc.Bacc("TRN2", target_bir_lowering=False, debug=False,
                   num_devices=N_CORES)

    def din(name, shape, dtype):
        return nc.dram_tensor(name, shape, dtype, kind="ExternalInput").ap()

    # Per-core sharded inputs. idx/val laid out [128, NTILE, NI] so tile t
    # is the partition-contiguous slice [:, t, :].
    idx_s = din("idx_s", [TILE, NTILE, NI], dt.int16)
    val_s = din("val_s", [TILE, NTILE, NI], dt.bfloat16)
    idx_n = din("idx_n", [TILE, NTILE, NI], dt.int16)
    val_n = din("val_n", [TILE, NTILE, NI], dt.bfloat16)
    # Replicated weights. wt = W_p.T reshaped [NQ, 128, FT] -> [128, NQ, FT].
    wt = din("wt", [128, NQ, FT], dt.bfloat16)
    wl2 = din("wl2", [128, NS, L2W], dt.bfloat16)   # W_l2.T as [128,8,32]
    wout = din("wout", [L2W, 1], dt.bfloat16)       # W_out.T
    bp = din("bp", [128, NA], dt.float32)           # b_p as [128,4]
    bl2 = din("bl2", [L2W, 1], dt.float32)
    bout = din("bout", [1, 1], dt.float32)
    ident = din("ident", [128, 128], dt.bfloat16)

    out_ap = nc.dram_tensor("out", [1, BL], dt.float32,
                            kind="ExternalOutput").ap()

    with tile.TileContext(nc) as tc:
        with (
            tc.tile_pool(name="const", bufs=1) as cpool,
            tc.tile_pool(name="hpool", bufs=3 * TPM) as hpool,
            tc.tile_pool(name="tq", bufs=10) as tqpool,
            tc.tile_pool(name="xpool", bufs=3) as xpool,
            tc.tile_pool(name="x2pool", bufs=2) as x2pool,
            tc.tile_pool(name="xcpool", bufs=4) as xcpool,
            tc.tile_pool(name="opool", bufs=2) as opool,
            tc.tile_pool(name="ps_t", bufs=3, space=bass.MemorySpace.PSUM) as ps_t,
            tc.tile_pool(name="ps_pov", bufs=2, space=bass.MemorySpace.PSUM) as ps_pov,
            tc.tile_pool(name="ps_head", bufs=1, space=bass.MemorySpace.PSUM) as ps_head,
        ):
            # --- warmup: pull the GPSIMD ext-isa library into IRAM and
            # the ACT LUTs before the first real dependency chain ---
            wu = cpool.tile([128, 2], dt.bfloat16, tag="wu")
            wu_i = cpool.tile([128, 2], dt.int16, tag="wu_i")
            nc.vector.memset(wu_i[:], -1)
            nc.vector.memset(wu[:], 0)
            nc.gpsimd.local_scatter(wu[:], wu[:], wu_i[:],
                                    channels=128, num_elems=2, num_idxs=2)
            wu_a = cpool.tile([128, 2], dt.float32, tag="wu_a")
            nc.scalar.activation(wu_a[:], wu[:],
                                 mybir.ActivationFunctionType.Relu)
            nc.scalar.activation(wu_a[:], wu[:],
                                 mybir.ActivationFunctionType.Sigmoid)
            # PE warm-up: ~3 us of dummy matmuls on scratch data releases
            # the HAM clock throttle before the first real matmul arrives.
            wu_m = cpool.tile([128, 512], dt.bfloat16, tag="wu_m")
            nc.vector.memset(wu_m[:], 0)
            wu_ps = ps_head.tile([128, 512], dt.float32,
                                 name="wups", tag="head")
            for i in range(14):
                nc.tensor.matmul(wu_ps[:], wu_m[:, 0:128], wu_m[:],
                                 start=True, stop=True)

            # --- load constants/weights/indices once. Launch order and
            # queue split matter: each DMA launch occupies its queue for
            # ~600 ns, and the identity + idx/val gate the first matmuls.
            id_t = cpool.tile([128, 128], dt.bfloat16, tag="ident")
            nc.scalar.dma_start(id_t[:], ident[:, :])
            idx_t = {}
            val_t = {}
            for pov, (ia, va) in enumerate(((idx_s, val_s), (idx_n, val_n))):
                idx_t[pov] = cpool.tile([TILE, NTILE, NI], dt.int16,
                                        name=f"idx{pov}", tag=f"idx{pov}")
                val_t[pov] = cpool.tile([TILE, NTILE, NI], dt.bfloat16,
                                        name=f"val{pov}", tag=f"val{pov}")
                if pov == 0:
                    nc.sync.dma_start(idx_t[pov][:, 0:1, :], ia[:, 0:1, :])
                    nc.sync.dma_start(val_t[pov][:, 0:1, :], va[:, 0:1, :])
                    nc.sync.dma_start(idx_t[pov][:, 1:, :], ia[:, 1:, :])
                    nc.sync.dma_start(val_t[pov][:, 1:, :], va[:, 1:, :])
                else:
                    nc.sync.dma_start(idx_t[pov][:], ia[:, :, :])
                    nc.sync.dma_start(val_t[pov][:], va[:, :, :])
            wt_t = cpool.tile([128, NQ, FT], dt.bfloat16, tag="wt")
            nc.scalar.dma_start(wt_t[:], wt[:, :, :])
            wl2_t = cpool.tile([128, NS, L2W], dt.bfloat16, tag="wl2")
            nc.scalar.dma_start(wl2_t[:], wl2[:, :, :])
            wout_t = cpool.tile([L2W, 1], dt.bfloat16, tag="wout")
            nc.scalar.dma_start(wout_t[:], wout[:, :])
            bp_t = cpool.tile([128, NA], dt.float32, tag="bp")
            nc.scalar.dma_start(bp_t[:], bp[:, :])
            bl2_t = cpool.tile([L2W, 1], dt.float32, tag="bl2")
            nc.scalar.dma_start(bl2_t[:], bl2[:, :])
            bout_t = cpool.tile([1, 1], dt.float32, tag="bout")
            nc.scalar.dma_start(bout_t[:], bout[:, :])

            # Processing units as (first 128-sample tile, tile count).
            UNITS = [(0, 4), (4, 4), (8, 4), (12, 4)]
            x_ts = {}

            def head(u):
                # L2 + output layers + sigmoid for unit u (runs deferred,
                # overlapped with the next unit's perspective work).
                tile0, ntiles = UNITS[u]
                w = ntiles * TILE
                x_t = x_ts.pop(u)
                l2_ps = ps_head.tile([L2W, MACRO], dt.float32,
                                     name="l2ps", tag="head")
                for s in range(NS):
                    nc.tensor.matmul(
                        l2_ps[:, 0:w], wl2_t[:, s, :], x_t[:, s, :],
                        start=(s == 0), stop=(s == NS - 1),
                    )
                x2_t = x2pool.tile([L2W, MACRO], dt.bfloat16, tag="x2")
                if ZERO_BL2:
                    nc.vector.tensor_scalar(
                        x2_t[:, 0:w], l2_ps[:, 0:w], 1.0, 0.0,
                        mybir.AluOpType.min, mybir.AluOpType.max,
                    )
                else:
                    nc.scalar.activation(
                        x2_t[:, 0:w], l2_ps[:, 0:w],
                        mybir.ActivationFunctionType.Relu,
                        bias=bl2_t[:, 0:1],
                    )
                    nc.vector.tensor_scalar_min(x2_t[:, 0:w],
                                                x2_t[:, 0:w], 1.0)
                o_ps = ps_head.tile([1, MACRO], dt.float32,
                                    name="ops", tag="head")
                nc.tensor.matmul(o_ps[:, 0:w], wout_t[:], x2_t[:, 0:w],
                                 start=True, stop=True)
                o_sb = opool.tile([1, MACRO], dt.float32, tag="osb")
                nc.scalar.activation(
                    o_sb[:, 0:w], o_ps[:, 0:w],
                    mybir.ActivationFunctionType.Sigmoid,
                    bias=bout_t[:, 0:1],
                )
                b0 = tile0 * TILE
                nc.sync.dma_start(out_ap[:, b0:b0 + w], o_sb[:, 0:w])

            for u, (tile0, ntiles) in enumerate(UNITS):
                w = ntiles * TILE
                # X holds the 8 activated k-slices for this unit:
                # slices 0..3 = stm perspective, 4..7 = nstm.
                x_t = xpool.tile([128, NS, MACRO], dt.bfloat16, tag="x")
                x_t = x_t[:, :, 0:w]
                x_ts[u] = x_t
                if u >= 1:
                    head(u - 1)
                for pov in range(2):
                    # --- densify: H[t] = board tile [128 samples, NF] ---
                    hts = []
                    for t in range(ntiles):
                        gt = tile0 + t
                        ht = hpool.tile([TILE, NF], dt.bfloat16,
                                        name=f"h{t}", tag=f"h{t}")
                        nc.gpsimd.local_scatter(
                            ht[:], val_t[pov][:, gt, :], idx_t[pov][:, gt, :],
                            channels=TILE, num_elems=NF, num_idxs=NI,
                        )
                        hts.append(ht)

                    # --- perspective layer: transpose each 128-wide
                    # feature chunk via identity matmul, cast to bf16
                    # (split across Scalar and Vector), then accumulate
                    # W_p.T in two 2-bank PSUM groups so consecutive units
                    # overlap.
                    t_sbs = []
                    for q in range(NQ):
                        t_ps = ps_t.tile([128, MACRO], dt.float32, tag="tps")
                        for t in range(ntiles):
                            nc.tensor.matmul(
                                t_ps[:, t * TILE:(t + 1) * TILE],
                                hts[t][:, q * 128:(q + 1) * 128],
                                id_t[:],
                                start=True, stop=True,
                            )
                        t_sb = tqpool.tile([128, MACRO], dt.bfloat16,
                                           name="tsb", tag="tsb")
                        if q % 2 == 0:
                            nc.scalar.copy(t_sb[:, 0:w], t_ps[:, 0:w])
                        else:
                            nc.vector.tensor_copy(t_sb[:, 0:w], t_ps[:, 0:w])
                        t_sbs.append(t_sb)

                    for g in range(2):
                        pv = {}
                        for al in range(2):
                            pv[al] = ps_pov.tile([128, MACRO], dt.float32,
                                                 name=f"pv{al}",
                                                 tag=f"pv{al}")
                        for q in range(NQ):
                            for al in range(2):
                                a = g * 2 + al
                                nc.tensor.matmul(
                                    pv[al][:, 0:w],
                                    wt_t[:, q, a * 128:(a + 1) * 128],
                                    t_sbs[q][:, 0:w],
                                    start=(q == 0), stop=(q == NQ - 1),
                                )
                        for al in range(2):
                            a = g * 2 + al
                            xs = x_t[:, pov * NA + a, :]
                            if ZERO_BP:
                                xc = xcpool.tile([128, MACRO], dt.bfloat16,
                                                 name=f"xc{al}",
                                                 tag=f"xc{al}")
                                nc.vector.tensor_scalar(
                                    xc[:, 0:w], pv[al][:, 0:w], 1.0, 0.0,
                                    mybir.AluOpType.min, mybir.AluOpType.max,
                                )
                                nc.scalar.activation(
                                    xs, xc[:, 0:w],
                                    mybir.ActivationFunctionType.Square,
                                )
                            else:
                                nc.scalar.activation(
                                    xs, pv[al][:, 0:w],
                                    mybir.ActivationFunctionType.Relu,
                                    bias=bp_t[:, a:a + 1],
                                )
                                nc.vector.tensor_scalar_min(xs, xs, 1.0)
                                nc.vector.tensor_mul(xs, xs, xs)

            head(len(UNITS) - 1)

    nc.compile()
    return nc


def _get_compiled(zero_bp, zero_bl2):
    key = (zero_bp, zero_bl2)
    if key not in _COMPILED:
        _COMPILED[key] = _build(zero_bp, zero_bl2)
    return _COMPILED[key]


def _dedup_rows(feats, vals):
    """Merge duplicate feature ids within each row.

    feats: [N, NI] int, vals: [N, NI] float32.
    Returns (idx int16 with -1 padding, val float32) where each feature id
    appears at most once per row with its values summed.
    """
    n = feats.shape[0]
    order = np.argsort(feats, axis=1, kind="stable")
    fs = np.take_along_axis(feats, order, 1)
    vs = np.take_along_axis(vals, order, 1)
    new_run = np.ones((n, NI), dtype=bool)
    new_run[:, 1:] = fs[:, 1:] != fs[:, :-1]
    run_id = np.cumsum(new_run, axis=1) - 1          # [N, NI] in [0, NI)
    rows = np.repeat(np.arange(n, dtype=np.int64), NI)
    key = rows * NI + run_id.ravel()
    totals = np.bincount(key, weights=vs.ravel().astype(np.float64),
                         minlength=n * NI).reshape(n, NI)
    idx = np.where(new_run, fs, -1).astype(np.int16)
    val = np.where(new_run, totals[np.arange(n)[:, None], run_id], 0.0)
    return idx, val.astype(np.float32)


def _prep_sparse(indices, values):
    """Interleaved (sample, feature) pairs -> per-core scatter inputs."""
    pairs = np.asarray(indices).astype(np.int64).reshape(-1, 2)
    vals = np.asarray(values, dtype=np.float32)
    samp = pairs[:, 0]
    expect = np.repeat(np.arange(B, dtype=np.int64), NI)
    if not np.array_equal(samp, expect):
        # general layout: stable-sort nnz by sample id first
        order = np.argsort(samp, kind="stable")
        pairs = pairs[order]
        vals = vals[order]
        assert np.array_equal(pairs[:, 0], expect), \
            "each sample must have exactly NI (sample, feature) pairs"
    feats = pairs[:, 1].reshape(B, NI)
    vals = vals.reshape(B, NI)
    idx, val = _dedup_rows(feats, vals)
    # [B, NI] -> per-core [TILE, NTILE, NI] with sample = t*TILE + p
    idx = idx.reshape(N_CORES, NTILE, TILE, NI).transpose(0, 2, 1, 3)
    val = val.reshape(N_CORES, NTILE, TILE, NI).transpose(0, 2, 1, 3)
    return (np.ascontiguousarray(idx),
            np.ascontiguousarray(val.astype(ml_dtypes.bfloat16)))


def kernel(stm_indices, nstm_indices, values, size,
           W_p, b_p, W_l2, b_l2, W_out, b_out):
    assert int(size) == B
    zero_bp = not np.any(np.asarray(b_p))
    zero_bl2 = not np.any(np.asarray(b_l2))
    nc = _get_compiled(zero_bp, zero_bl2)

    idx_s, val_s = _prep_sparse(stm_indices, values)
    idx_n, val_n = _prep_sparse(nstm_indices, values)

    bf16 = ml_dtypes.bfloat16
    W_p = np.asarray(W_p, dtype=np.float32)       # [FT, NF]
    wt = np.ascontiguousarray(
        W_p.T.reshape(NQ, 128, FT).transpose(1, 0, 2).astype(bf16))
    W_l2 = np.asarray(W_l2, dtype=np.float32)     # [L2W, 2*FT]
    wl2 = np.ascontiguousarray(
        W_l2.T.reshape(NS, 128, L2W).transpose(1, 0, 2).astype(bf16))
    wout = np.ascontiguousarray(
        np.asarray(W_out, dtype=np.float32).T.astype(bf16))  # [32,1]
    bp = np.ascontiguousarray(
        np.asarray(b_p, dtype=np.float32).reshape(NA, 128).T)
    bl2 = np.asarray(b_l2, dtype=np.float32).reshape(L2W, 1)
    bout = np.asarray(b_out, dtype=np.float32).reshape(1, 1)
    ident = np.eye(128, dtype=bf16)

    in_maps = []
    for c in range(N_CORES):
        in_maps.append({
            "idx_s": idx_s[c], "val_s": val_s[c],
            "idx_n": idx_n[c], "val_n": val_n[c],
            "wt": wt, "wl2": wl2, "wout": wout,
            "bp": bp, "bl2": bl2, "bout": bout,
            "ident": ident,
        })

    res = run_bass_kernel_spmd(nc, in_maps, core_ids=list(range(N_CORES)),
                               **RUN_KWARGS)
    global LAST_RESULT
    LAST_RESULT = res
    out = np.concatenate([res.results[c]["out"].reshape(BL)
                          for c in range(N_CORES)])
    return out.reshape(B, 1).astype(np.float32)
